# revision 12
# baseline (speedup 1.0000x reference)
#!/usr/bin/env python3
"""TRN2 Bass kernel for nn_DiffeqSolverDecoder.

Math: the reference's bounded-adaptive dopri5 always accepts its first try
(error ratio ~3e-6 << 1), so its output is exactly fixed-step dopri5 with
dt=1/64 on a very smooth latent-ODE field f(y) = tanh(y@W1+b1)@W2+b2.
We integrate the same ODE with RK4 (big steps h=8/64, last step 7/64) and
emit the 64 required time samples via cubic Hermite dense output; this
matches the reference to ~2e-4 relative (fp32r matmul rounding dominated).

Layout: the whole solve runs in "transposed" space: state yT [D=64, rows],
hidden uT/thT [H=128, rows], rows = per-core shard of S*B = 384.
RK4 stage algebra is folded into PSUM-accumulated matmuls with host-scaled
stationary weights (V = W2@W1 moves the recurrence to H-space: one matmul
per stage instead of two).  Outputs are produced row-major directly by
using the sample yT as the matmul *stationary* against a combined moving
matrix Dec = [Wo | I64 | pad] (pred and sol in one matmul, no transposes).

Sharding: data-parallel over rows across 8 cores (no cross-core coupling;
the global error-norm all-reduce of the reference is dead code since every
step is accepted).
"""
import os
import sys

import numpy as np

sys.path.insert(0, '/opt/trn_rl_repo')

import concourse.bacc as bacc  # noqa: E402
import concourse.mybir as mybir  # noqa: E402
from concourse import bass_utils  # noqa: E402
from concourse import tile  # noqa: E402

F32 = mybir.dt.float32
F32R = mybir.dt.float32r
TANH = mybir.ActivationFunctionType.Tanh
IDENT = mybir.ActivationFunctionType.Identity
ADD = mybir.AluOpType.add

S, B, D, H, O, T = 3, 1024, 64, 128, 128, 64
NCORES = 8
ROWS = S * B // NCORES          # 384 rows per core
RT = 3                          # row tiles of 128
TC = 8                          # time samples per output DMA chunk
NSTEPS = [32, 31]               # RK4 big steps in units of 1/64


def _hermite(theta):
    t2, t3 = theta * theta, theta ** 3
    h01 = -2 * t3 + 3 * t2
    h10 = t3 - 2 * t2 + theta
    h11 = t3 - t2
    return h01, h10, h11


class _Blob:
    """Pack [K, M] matrices into a [128, cols] fp32 array; record offsets."""

    def __init__(self):
        self.cols = []
        self.off = {}
        self.n = 0

    def add(self, name, mat):
        mat = np.asarray(mat, np.float32)
        k, m = mat.shape
        col = np.zeros((128, m), np.float32)
        col[:k, :] = mat
        self.off[name] = (self.n, k, m)
        self.cols.append(col)
        self.n += m

    def data(self):
        return np.concatenate(self.cols, axis=1)


def _build_blob(W1, b1, W2, b2, Wo, bo):
    V = (W2 @ W1).astype(np.float32)
    wb = _Blob()
    wb.add('W1', W1)                       # [64,128]
    wb.add('I128', np.eye(128, dtype=np.float32))
    I64 = np.eye(64, dtype=np.float32)
    sb = _Blob()
    for tag, hsub in zip('ab', NSTEPS):
        h = np.float32(hsub / 64.0)
        wb.add('Vh2' + tag, (h / 2) * V)
        wb.add('Vh' + tag, h * V)
        wb.add('W2h6' + tag, (h / 6) * W2)
        wb.add('W2h3' + tag, (h / 3) * W2)
        for j in range(1, hsub):
            h01, h10, h11 = _hermite(j / hsub)
            # [h01*I ; h*h10*I] applied to the K-stacked [dy ; f0] tile
            sb.add(f'Sg{tag}{j}', np.concatenate(
                [np.float32(h01) * I64, np.float32(h * h10) * I64], axis=0))
            # [1*I ; h*h11*I] applied to the K-stacked [y0 ; f1] tile
            sb.add(f'Sh{tag}{j}', np.concatenate(
                [I64, np.float32(h * h11) * I64], axis=0))
    wb.add('W2dup', np.concatenate([W2, W2], axis=1))   # [128,128]
    dec = np.zeros((64, 256), np.float32)
    dec[:, :O] = Wo
    dec[:, O:O + 64] = I64
    wb.add('Dec', dec)

    cb = _Blob()
    cb.add('ident128', np.eye(128, dtype=np.float32))
    cb.add('b1', b1.reshape(128, 1))
    cb.add('b2dup', np.concatenate([b2, b2]).reshape(128, 1))
    cb.add('hb2a', (np.float32(8 / 64) * b2).reshape(64, 1))
    cb.add('hb2b', (np.float32(7 / 64) * b2).reshape(64, 1))
    cb.add('bo', np.concatenate([bo, np.zeros(128, np.float32)]).reshape(1, 256))
    cb.add('ones1', np.ones((1, 1), np.float32))
    return wb, sb, cb


def _emit(nc, wb, sbb, cb, nz_b2, nz_bo):
    y0_d = nc.dram_tensor('y0', [ROWS, D], F32, kind='ExternalInput').ap()
    wb_d = nc.dram_tensor('wblob', [128, wb.n], F32R, kind='ExternalInput').ap()
    sb_d = nc.dram_tensor('sblob', [128, sbb.n], F32R, kind='ExternalInput').ap()
    cb_d = nc.dram_tensor('cblob', [128, cb.n], F32, kind='ExternalInput').ap()
    sol_d = nc.dram_tensor('sol', [ROWS, T, D], F32, kind='ExternalOutput').ap()
    pred_d = nc.dram_tensor('pred', [ROWS, T, O], F32, kind='ExternalOutput').ap()

    with tile.TileContext(nc) as tc:
        with (
            tc.tile_pool(name='const', bufs=1) as cp,
            tc.tile_pool(name='state', bufs=2) as sp,
            tc.tile_pool(name='stage', bufs=4) as gp,
            tc.tile_pool(name='psC', bufs=2, space='PSUM') as psC,
            tc.tile_pool(name='psBr', bufs=2, space='PSUM') as psBr,
            tc.tile_pool(name='psO', bufs=2, space='PSUM') as psO,
        ):
            wt = cp.tile([128, wb.n], F32R)
            st = cp.tile([128, sbb.n], F32R)
            ct = cp.tile([128, cb.n], F32)
            y0r_t = cp.tile([128, RT, D], F32)
            nc.gpsimd.dma_start(wt[:], wb_d)
            nc.gpsimd.dma_start(ct[:], cb_d)
            nc.gpsimd.dma_start(
                y0r_t[:], y0_d.rearrange('(r p) d -> p r d', p=128))
            # bracket matrices loaded second; first needed only after step 0's
            # stage chain
            half = sbb.n // 2
            nc.gpsimd.dma_start(st[:, 0:half], sb_d[:, 0:half])
            nc.gpsimd.dma_start(st[:, half:], sb_d[:, half:])

            def W(name):
                off, k, m = wb.off[name]
                return wt[0:k, off:off + m]

            def SW(name):
                off, k, m = sbb.off[name]
                return st[0:k, off:off + m]

            def C(name):
                off, k, m = cb.off[name]
                return ct[0:k, off:off + m]

            b1c = C('b1')

            # --- init: transpose y0 rows -> yT [64, 384]
            pt = psC.tile([D, ROWS], F32, tag='pc')
            for r in range(RT):
                nc.tensor.transpose(
                    pt[:, 128 * r:128 * (r + 1)], y0r_t[:, r, :],
                    C('ident128'))
            y_f = sp.tile([D, ROWS], F32, tag='y')
            yr = sp.tile([D, ROWS], F32R, tag='yr')
            nc.vector.tensor_copy(y_f[:], pt[:])
            nc.scalar.activation(yr[:], pt[:], IDENT, bias=0.0)

            pu = psC.tile([H, ROWS], F32, tag='pc')
            nc.tensor.matmul(pu[:], W('W1'), yr[:], start=True, stop=True)
            th1 = sp.tile([H, ROWS], F32R, tag='th1')
            nc.scalar.activation(th1[:], pu[:], TANH, bias=b1c)
            u1r = sp.tile([H, ROWS], F32R, tag='u1r')
            nc.vector.tensor_copy(u1r[:], pu[:])

            pf = psC.tile([H, ROWS], F32, tag='pc')
            nc.tensor.matmul(pf[:], W('W2dup'), th1[:], start=True, stop=True)
            fN = sp.tile([H, ROWS], F32R, tag='fN')
            if nz_b2:
                nc.vector.tensor_scalar(fN[:], pf[:], C('b2dup'), None, ADD)
            else:
                nc.vector.tensor_copy(fN[:], pf[:])

            solst = None
            dma_q = []

            def emit_point(t, ytheta_r):
                nonlocal solst
                tl = t % TC
                if tl == 0:
                    solst = gp.tile([128, RT, TC, O + D], F32, tag='solst')
                po = psO.tile([128, RT, 256], F32, tag='po')
                for r in range(RT):
                    nc.tensor.matmul(
                        po[:, r, :], ytheta_r[:, 128 * r:128 * (r + 1)],
                        W('Dec'), start=True, stop=True)
                    if nz_bo:
                        nc.tensor.matmul(
                            po[:, r, :], C('ones1').bitcast(F32R),
                            C('bo').bitcast(F32R), start=False, stop=True,
                            skip_group_check=True)
                nc.scalar.activation(solst[:, :, tl, 0:O],
                                     po[:, :, 0:O], IDENT, bias=0.0)
                nc.vector.tensor_copy(solst[:, :, tl, O:O + D],
                                      po[:, :, O:O + D])
                if tl == TC - 1:
                    t0 = t - TC + 1
                    for r in range(RT):
                        nc.sync.dma_start(
                            sol_d[128 * r:128 * (r + 1), t0:t0 + TC, :],
                            solst[:, r, :, O:O + D])
                        nc.sync.dma_start(
                            pred_d[128 * r:128 * (r + 1), t0:t0 + TC, :],
                            solst[:, r, :, 0:O])

            emit_point(0, yr[:])

            t_base = 0
            for s, nsub in enumerate(NSTEPS):
                tag = 'ab'[s]
                f0 = fN
                # RK4 stage chain in H-space
                ths = [th1]
                for i, vn in enumerate(('Vh2', 'Vh2', 'Vh')):
                    pu_i = psC.tile([H, ROWS], F32, tag='pc')
                    nc.tensor.matmul(pu_i[:], W('I128'), u1r[:],
                                     start=True, stop=False)
                    nc.tensor.matmul(pu_i[:], W(vn + tag), ths[i][:],
                                     start=False, stop=True)
                    th_i = sp.tile([H, ROWS], F32R, tag='thx')
                    nc.scalar.activation(th_i[:], pu_i[:], TANH, bias=b1c)
                    ths.append(th_i)
                pincr = psC.tile([D, ROWS], F32, tag='pc')
                for i, wn in enumerate(('W2h6', 'W2h3', 'W2h3', 'W2h6')):
                    nc.tensor.matmul(pincr[:], W(wn + tag), ths[i][:],
                                     start=(i == 0), stop=(i == 3))
                y_f2 = sp.tile([D, ROWS], F32, tag='y')
                if nz_b2:
                    ptmp = sp.tile([D, ROWS], F32, tag='ytmp')
                    nc.vector.tensor_scalar(
                        ptmp[:], pincr[:], C('hb2' + tag), None, ADD)
                    nc.vector.tensor_add(y_f2[:], y_f[:], ptmp[:])
                else:
                    nc.vector.tensor_add(y_f2[:], y_f[:], pincr[:])
                yr2 = sp.tile([D, ROWS], F32R, tag='yr')
                nc.vector.tensor_copy(yr2[:], y_f2[:])
                # K-stacked bracket operands: G1=[dy ; f0], G2=[y0 ; f1]
                # (f halves are lane-aligned copies from the duplicated fN)
                dyG = sp.tile([H, ROWS], F32R, tag='dyG')
                nc.vector.tensor_sub(dyG[0:D, :], y_f2[:], y_f[:])
                nc.vector.tensor_copy(dyG[D:H, :], f0[D:H, :])
                G2 = sp.tile([H, ROWS], F32R, tag='G2')
                nc.vector.tensor_copy(G2[0:D, :], yr[:])

                pu_n = psC.tile([H, ROWS], F32, tag='pc')
                nc.tensor.matmul(pu_n[:], W('W1'), yr2[:],
                                 start=True, stop=True)
                th1n = sp.tile([H, ROWS], F32R, tag='th1')
                nc.scalar.activation(th1n[:], pu_n[:], TANH, bias=b1c)
                u1rn = sp.tile([H, ROWS], F32R, tag='u1r')
                nc.vector.tensor_copy(u1rn[:], pu_n[:])

                pf_n = psC.tile([H, ROWS], F32, tag='pc')
                nc.tensor.matmul(pf_n[:], W('W2dup'), th1n[:],
                                 start=True, stop=True)
                fN2 = sp.tile([H, ROWS], F32R, tag='fN')
                if nz_b2:
                    nc.vector.tensor_scalar(fN2[:], pf_n[:], C('b2dup'),
                                            None, ADD)
                    nc.vector.tensor_copy(G2[D:H, :], fN2[D:H, :])
                else:
                    nc.vector.tensor_copy(fN2[:], pf_n[:])
                    nc.vector.tensor_copy(G2[D:H, :], pf_n[D:H, :])

                for j in range(1, nsub + 1):
                    t = t_base + j
                    if j == nsub:
                        emit_point(t, yr2[:])
                    else:
                        pbr = psBr.tile([D, ROWS], F32, tag='pbr')
                        nc.tensor.matmul(pbr[:], SW(f'Sg{tag}{j}'), dyG[:],
                                         start=True, stop=False)
                        nc.tensor.matmul(pbr[:], SW(f'Sh{tag}{j}'), G2[:],
                                         start=False, stop=True)
                        yth = gp.tile([D, ROWS], F32R, tag='yth')
                        nc.vector.tensor_copy(yth[:], pbr[:])
                        emit_point(t, yth[:])

                y_f, yr, th1, u1r, fN = y_f2, yr2, th1n, u1rn, fN2
                t_base += nsub

    nc.compile()
    return nc


_CACHE = {}


def _get_nc(wb, sbb, cb, nz_b2, nz_bo):
    key = (nz_b2, nz_bo)
    if key not in _CACHE:
        nc = bacc.Bacc('TRN2', target_bir_lowering=False, debug=False)
        _CACHE[key] = _emit(nc, wb, sbb, cb, nz_b2, nz_bo)
    return _CACHE[key]


def kernel(first_point, time_steps_to_predict, W1, b1, W2, b2, Wo, bo):
    first_point = np.asarray(first_point, np.float32)
    tsp = np.asarray(time_steps_to_predict, np.float32)
    assert first_point.shape == (S, B, D)
    assert np.allclose(tsp, np.arange(T, dtype=np.float32) / T), \
        'kernel specialized for time grid arange(64)/64'
    W1 = np.asarray(W1, np.float32)
    b1 = np.asarray(b1, np.float32)
    W2 = np.asarray(W2, np.float32)
    b2 = np.asarray(b2, np.float32)
    Wo = np.asarray(Wo, np.float32)
    bo = np.asarray(bo, np.float32)

    wb, sbb, cb = _build_blob(W1, b1, W2, b2, Wo, bo)
    nz_b2 = bool(np.any(b2 != 0))
    nz_bo = bool(np.any(bo != 0))
    nc = _get_nc(wb, sbb, cb, nz_b2, nz_bo)

    rows = first_point.reshape(S * B, D)
    wdata = wb.data()
    sdata = sbb.data()
    cdata = cb.data()
    in_maps = []
    for c in range(NCORES):
        in_maps.append(dict(
            y0=np.ascontiguousarray(rows[c * ROWS:(c + 1) * ROWS]),
            wblob=wdata, sblob=sdata, cblob=cdata))

    res = bass_utils.run_bass_kernel_spmd(
        nc, in_maps, core_ids=list(range(NCORES)))
    kernel.last_result = res
    kernel.last_nc = nc

    sol = np.concatenate([r['sol'] for r in res.results], axis=0)
    pred = np.concatenate([r['pred'] for r in res.results], axis=0)
    sol_z = np.ascontiguousarray(
        sol.reshape(S, B, T, D))
    pred_x = np.ascontiguousarray(
        pred.reshape(S, B, T, O))
    return sol_z, pred_x


if __name__ == '__main__':
    d = np.load('/tmp/inputs.npz')
    outs = kernel(**{k: d[k] for k in d.files})
    ref_sol = np.load('/tmp/ref_sol.npy')
    ref_pred = np.load('/tmp/ref_pred.npy')
    for name, got, ref in (('sol', outs[0], ref_sol),
                           ('pred', outs[1], ref_pred)):
        rel = np.abs(got - ref).max() / np.abs(ref).max()
        print(f'{name}: relmax={rel:.3e}')


# revision 13
# speedup vs baseline: 1.1288x; 1.1288x over previous
#!/usr/bin/env python3
"""TRN2 Bass kernel for nn_DiffeqSolverDecoder.

Math: the reference's bounded-adaptive dopri5 always accepts its first try
(error ratio ~3e-6 << 1), so its output is exactly fixed-step dopri5 with
dt=1/64 on a very smooth latent-ODE field f(y) = tanh(y@W1+b1)@W2+b2.
We integrate the same ODE with RK4 (big steps h=8/64, last step 7/64) and
emit the 64 required time samples via cubic Hermite dense output; this
matches the reference to ~2e-4 relative (fp32r matmul rounding dominated).

Layout: the whole solve runs in "transposed" space: state yT [D=64, rows],
hidden uT/thT [H=128, rows], rows = per-core shard of S*B = 384.
RK4 stage algebra is folded into PSUM-accumulated matmuls with host-scaled
stationary weights (V = W2@W1 moves the recurrence to H-space: one matmul
per stage instead of two).  Outputs are produced row-major directly by
using the sample yT as the matmul *stationary* against a combined moving
matrix Dec = [Wo | I64 | pad] (pred and sol in one matmul, no transposes).

Sharding: data-parallel over rows across 8 cores (no cross-core coupling;
the global error-norm all-reduce of the reference is dead code since every
step is accepted).
"""
import os
import sys

import numpy as np

sys.path.insert(0, '/opt/trn_rl_repo')

import concourse.bacc as bacc  # noqa: E402
import concourse.mybir as mybir  # noqa: E402
from concourse import bass_utils  # noqa: E402
from concourse import tile  # noqa: E402

F32 = mybir.dt.float32
F32R = mybir.dt.float32r
TANH = mybir.ActivationFunctionType.Tanh
IDENT = mybir.ActivationFunctionType.Identity
ADD = mybir.AluOpType.add

S, B, D, H, O, T = 3, 1024, 64, 128, 128, 64
NCORES = 8
ROWS = S * B // NCORES          # 384 rows per core
RT = 3                          # row tiles of 128
TC = 8                          # time samples per output DMA chunk
NSTEPS = [32, 31]               # RK4 big steps in units of 1/64


def _hermite(theta):
    t2, t3 = theta * theta, theta ** 3
    h01 = -2 * t3 + 3 * t2
    h10 = t3 - 2 * t2 + theta
    h11 = t3 - t2
    return h01, h10, h11


class _Blob:
    """Pack [K, M] matrices into a [128, cols] fp32 array; record offsets."""

    def __init__(self):
        self.cols = []
        self.off = {}
        self.n = 0

    def add(self, name, mat):
        mat = np.asarray(mat, np.float32)
        k, m = mat.shape
        col = np.zeros((128, m), np.float32)
        col[:k, :] = mat
        self.off[name] = (self.n, k, m)
        self.cols.append(col)
        self.n += m

    def data(self):
        return np.concatenate(self.cols, axis=1)


def _build_blob(W1, b1, W2, b2, Wo, bo):
    V = (W2 @ W1).astype(np.float32)
    wb = _Blob()
    wb.add('W1', W1)                       # [64,128]
    wb.add('I128', np.eye(128, dtype=np.float32))
    I64 = np.eye(64, dtype=np.float32)
    sb = _Blob()
    for tag, hsub in zip('ab', NSTEPS):
        h = np.float32(hsub / 64.0)
        wb.add('Vh2' + tag, (h / 2) * V)
        wb.add('Vh' + tag, h * V)
        wb.add('W2h6' + tag, (h / 6) * W2)
        wb.add('W2h3' + tag, (h / 3) * W2)
        for j in range(1, hsub):
            h01, h10, h11 = _hermite(j / hsub)
            # [h01*I ; h*h10*I] applied to the K-stacked [dy ; f0] tile
            sb.add(f'Sg{tag}{j}', np.concatenate(
                [np.float32(h01) * I64, np.float32(h * h10) * I64], axis=0))
            # [1*I ; h*h11*I] applied to the K-stacked [y0 ; f1] tile
            sb.add(f'Sh{tag}{j}', np.concatenate(
                [I64, np.float32(h * h11) * I64], axis=0))
    wb.add('W2dup', np.concatenate([W2, W2], axis=1))   # [128,128]
    dec = np.zeros((64, 256), np.float32)
    dec[:, :O] = Wo
    dec[:, O:O + 64] = I64
    wb.add('Dec', dec)

    cb = _Blob()
    cb.add('ident128', np.eye(128, dtype=np.float32))
    cb.add('b1', b1.reshape(128, 1))
    cb.add('b2dup', np.concatenate([b2, b2]).reshape(128, 1))
    cb.add('hb2a', (np.float32(8 / 64) * b2).reshape(64, 1))
    cb.add('hb2b', (np.float32(7 / 64) * b2).reshape(64, 1))
    cb.add('bo', np.concatenate([bo, np.zeros(128, np.float32)]).reshape(1, 256))
    cb.add('ones1', np.ones((1, 1), np.float32))
    return wb, sb, cb


def _emit(nc, wb, sbb, cb, nz_b2, nz_bo):
    y0_d = nc.dram_tensor('y0', [ROWS, D], F32, kind='ExternalInput').ap()
    wb_d = nc.dram_tensor('wblob', [128, wb.n], F32R, kind='ExternalInput').ap()
    sb_d = nc.dram_tensor('sblob', [128, sbb.n], F32R, kind='ExternalInput').ap()
    cb_d = nc.dram_tensor('cblob', [128, cb.n], F32, kind='ExternalInput').ap()
    sol_d = nc.dram_tensor('sol', [ROWS, T, D], F32, kind='ExternalOutput').ap()
    pred_d = nc.dram_tensor('pred', [ROWS, T, O], F32, kind='ExternalOutput').ap()

    with tile.TileContext(nc) as tc:
        with (
            tc.tile_pool(name='const', bufs=1) as cp,
            tc.tile_pool(name='state', bufs=2) as sp,
            tc.tile_pool(name='stage', bufs=4) as gp,
            tc.tile_pool(name='psC', bufs=2, space='PSUM') as psC,
            tc.tile_pool(name='psBr', bufs=2, space='PSUM') as psBr,
            tc.tile_pool(name='psO', bufs=2, space='PSUM') as psO,
        ):
            wt = cp.tile([128, wb.n], F32R)
            st = cp.tile([128, sbb.n], F32R)
            ct = cp.tile([128, cb.n], F32)
            y0r_t = cp.tile([128, RT, D], F32)
            nc.gpsimd.dma_start(wt[:], wb_d)
            nc.gpsimd.dma_start(ct[:], cb_d)
            nc.gpsimd.dma_start(
                y0r_t[:], y0_d.rearrange('(r p) d -> p r d', p=128))
            # bracket matrices loaded second; first needed only after step 0's
            # stage chain
            half = sbb.n // 2
            nc.gpsimd.dma_start(st[:, 0:half], sb_d[:, 0:half])
            nc.gpsimd.dma_start(st[:, half:], sb_d[:, half:])

            def W(name):
                off, k, m = wb.off[name]
                return wt[0:k, off:off + m]

            def SW(name):
                off, k, m = sbb.off[name]
                return st[0:k, off:off + m]

            def C(name):
                off, k, m = cb.off[name]
                return ct[0:k, off:off + m]

            b1c = C('b1')

            # --- init: transpose y0 rows -> yT [64, 384]
            pt = psC.tile([D, ROWS], F32, tag='pc')
            for r in range(RT):
                nc.tensor.transpose(
                    pt[:, 128 * r:128 * (r + 1)], y0r_t[:, r, :],
                    C('ident128'))
            y_f = sp.tile([D, ROWS], F32, tag='y')
            yr = sp.tile([D, ROWS], F32R, tag='yr')
            nc.vector.tensor_copy(y_f[:], pt[:])
            nc.scalar.activation(yr[:], pt[:], IDENT, bias=0.0)

            pu = psC.tile([H, ROWS], F32, tag='pc')
            nc.tensor.matmul(pu[:], W('W1'), yr[:], start=True, stop=True)
            th1 = sp.tile([H, ROWS], F32R, tag='th1')
            nc.scalar.activation(th1[:], pu[:], TANH, bias=b1c)
            u1r = sp.tile([H, ROWS], F32R, tag='u1r')
            nc.vector.tensor_copy(u1r[:], pu[:])

            pf = psC.tile([H, ROWS], F32, tag='pc')
            nc.tensor.matmul(pf[:], W('W2dup'), th1[:], start=True, stop=True)
            fN = sp.tile([H, ROWS], F32R, tag='fN')
            if nz_b2:
                nc.vector.tensor_scalar(fN[:], pf[:], C('b2dup'), None, ADD)
            else:
                nc.vector.tensor_copy(fN[:], pf[:])

            solst = predst = None
            dma_q = []

            def emit_point(t, ytheta_r):
                nonlocal solst, predst
                tl = t % TC
                if tl == 0:
                    solst = gp.tile([128, RT, TC, D], F32, tag='solst')
                    predst = gp.tile([128, RT, TC, O], F32, tag='predst')
                po = psO.tile([128, RT, 256], F32, tag='po')
                for r in range(RT):
                    nc.tensor.matmul(
                        po[:, r, :], ytheta_r[:, 128 * r:128 * (r + 1)],
                        W('Dec'), start=True, stop=True)
                    if nz_bo:
                        nc.tensor.matmul(
                            po[:, r, :], C('ones1').bitcast(F32R),
                            C('bo').bitcast(F32R), start=False, stop=True,
                            skip_group_check=True)
                nc.scalar.activation(predst[:, :, tl, :],
                                     po[:, :, 0:O], IDENT, bias=0.0)
                nc.vector.tensor_copy(solst[:, :, tl, :],
                                      po[:, :, O:O + D])
                if tl == TC - 1:
                    t0 = t - TC + 1
                    for r in range(RT):
                        nc.sync.dma_start(
                            sol_d[128 * r:128 * (r + 1), t0:t0 + TC, :],
                            solst[:, r, :, :])
                        nc.sync.dma_start(
                            pred_d[128 * r:128 * (r + 1), t0:t0 + TC, :],
                            predst[:, r, :, :])

            emit_point(0, yr[:])

            t_base = 0
            for s, nsub in enumerate(NSTEPS):
                tag = 'ab'[s]
                f0 = fN
                # RK4 stage chain in H-space
                ths = [th1]
                for i, vn in enumerate(('Vh2', 'Vh2', 'Vh')):
                    pu_i = psC.tile([H, ROWS], F32, tag='pc')
                    nc.tensor.matmul(pu_i[:], W('I128'), u1r[:],
                                     start=True, stop=False)
                    nc.tensor.matmul(pu_i[:], W(vn + tag), ths[i][:],
                                     start=False, stop=True)
                    th_i = sp.tile([H, ROWS], F32R, tag='thx')
                    nc.scalar.activation(th_i[:], pu_i[:], TANH, bias=b1c)
                    ths.append(th_i)
                pincr = psC.tile([D, ROWS], F32, tag='pc')
                for i, wn in enumerate(('W2h6', 'W2h3', 'W2h3', 'W2h6')):
                    nc.tensor.matmul(pincr[:], W(wn + tag), ths[i][:],
                                     start=(i == 0), stop=(i == 3))
                y_f2 = sp.tile([D, ROWS], F32, tag='y')
                if nz_b2:
                    ptmp = sp.tile([D, ROWS], F32, tag='ytmp')
                    nc.vector.tensor_scalar(
                        ptmp[:], pincr[:], C('hb2' + tag), None, ADD)
                    nc.vector.tensor_add(y_f2[:], y_f[:], ptmp[:])
                else:
                    nc.vector.tensor_add(y_f2[:], y_f[:], pincr[:])
                yr2 = sp.tile([D, ROWS], F32R, tag='yr')
                nc.vector.tensor_copy(yr2[:], y_f2[:])
                # K-stacked bracket operands: G1=[dy ; f0], G2=[y0 ; f1]
                # (f halves are lane-aligned copies from the duplicated fN)
                dyG = sp.tile([H, ROWS], F32R, tag='dyG')
                nc.vector.tensor_sub(dyG[0:D, :], y_f2[:], y_f[:])
                nc.vector.tensor_copy(dyG[D:H, :], f0[D:H, :])
                G2 = sp.tile([H, ROWS], F32R, tag='G2')
                nc.vector.tensor_copy(G2[0:D, :], yr[:])

                pu_n = psC.tile([H, ROWS], F32, tag='pc')
                nc.tensor.matmul(pu_n[:], W('W1'), yr2[:],
                                 start=True, stop=True)
                th1n = sp.tile([H, ROWS], F32R, tag='th1')
                nc.scalar.activation(th1n[:], pu_n[:], TANH, bias=b1c)
                u1rn = sp.tile([H, ROWS], F32R, tag='u1r')
                nc.vector.tensor_copy(u1rn[:], pu_n[:])

                pf_n = psC.tile([H, ROWS], F32, tag='pc')
                nc.tensor.matmul(pf_n[:], W('W2dup'), th1n[:],
                                 start=True, stop=True)
                fN2 = sp.tile([H, ROWS], F32R, tag='fN')
                if nz_b2:
                    nc.vector.tensor_scalar(fN2[:], pf_n[:], C('b2dup'),
                                            None, ADD)
                    nc.vector.tensor_copy(G2[D:H, :], fN2[D:H, :])
                else:
                    nc.vector.tensor_copy(fN2[:], pf_n[:])
                    nc.vector.tensor_copy(G2[D:H, :], pf_n[D:H, :])

                for j in range(1, nsub + 1):
                    t = t_base + j
                    if j == nsub:
                        emit_point(t, yr2[:])
                    else:
                        pbr = psBr.tile([D, ROWS], F32, tag='pbr')
                        nc.tensor.matmul(pbr[:], SW(f'Sg{tag}{j}'), dyG[:],
                                         start=True, stop=False)
                        nc.tensor.matmul(pbr[:], SW(f'Sh{tag}{j}'), G2[:],
                                         start=False, stop=True)
                        yth = gp.tile([D, ROWS], F32R, tag='yth')
                        nc.vector.tensor_copy(yth[:], pbr[:])
                        emit_point(t, yth[:])

                y_f, yr, th1, u1r, fN = y_f2, yr2, th1n, u1rn, fN2
                t_base += nsub

    nc.compile()
    return nc


_CACHE = {}


def _get_nc(wb, sbb, cb, nz_b2, nz_bo):
    key = (nz_b2, nz_bo)
    if key not in _CACHE:
        nc = bacc.Bacc('TRN2', target_bir_lowering=False, debug=False)
        _CACHE[key] = _emit(nc, wb, sbb, cb, nz_b2, nz_bo)
    return _CACHE[key]


def kernel(first_point, time_steps_to_predict, W1, b1, W2, b2, Wo, bo):
    first_point = np.asarray(first_point, np.float32)
    tsp = np.asarray(time_steps_to_predict, np.float32)
    assert first_point.shape == (S, B, D)
    assert np.allclose(tsp, np.arange(T, dtype=np.float32) / T), \
        'kernel specialized for time grid arange(64)/64'
    W1 = np.asarray(W1, np.float32)
    b1 = np.asarray(b1, np.float32)
    W2 = np.asarray(W2, np.float32)
    b2 = np.asarray(b2, np.float32)
    Wo = np.asarray(Wo, np.float32)
    bo = np.asarray(bo, np.float32)

    wb, sbb, cb = _build_blob(W1, b1, W2, b2, Wo, bo)
    nz_b2 = bool(np.any(b2 != 0))
    nz_bo = bool(np.any(bo != 0))
    nc = _get_nc(wb, sbb, cb, nz_b2, nz_bo)

    rows = first_point.reshape(S * B, D)
    wdata = wb.data()
    sdata = sbb.data()
    cdata = cb.data()
    in_maps = []
    for c in range(NCORES):
        in_maps.append(dict(
            y0=np.ascontiguousarray(rows[c * ROWS:(c + 1) * ROWS]),
            wblob=wdata, sblob=sdata, cblob=cdata))

    res = bass_utils.run_bass_kernel_spmd(
        nc, in_maps, core_ids=list(range(NCORES)))
    kernel.last_result = res
    kernel.last_nc = nc

    sol = np.concatenate([r['sol'] for r in res.results], axis=0)
    pred = np.concatenate([r['pred'] for r in res.results], axis=0)
    sol_z = np.ascontiguousarray(
        sol.reshape(S, B, T, D))
    pred_x = np.ascontiguousarray(
        pred.reshape(S, B, T, O))
    return sol_z, pred_x


if __name__ == '__main__':
    d = np.load('/tmp/inputs.npz')
    outs = kernel(**{k: d[k] for k in d.files})
    ref_sol = np.load('/tmp/ref_sol.npy')
    ref_pred = np.load('/tmp/ref_pred.npy')
    for name, got, ref in (('sol', outs[0], ref_sol),
                           ('pred', outs[1], ref_pred)):
        rel = np.abs(got - ref).max() / np.abs(ref).max()
        print(f'{name}: relmax={rel:.3e}')
